# revision 21
# baseline (speedup 1.0000x reference)
"""Trainium2 Bass kernel for nn_Concentration_61229053772314.

kernel(**inputs) takes the FULL inputs (B=64), shards the batch dim across
8 NeuronCores (pure data parallel, weights replicated), runs a Bass/Tile
kernel via run_bass_kernel_spmd, and reassembles the full outputs.

v5 architecture (per core: NBA=256 (b,a) rows, 4 groups of GRP=64):
 - ve streamed once as f32 in parity layout [p, (b, j, h)] with n = 2p+j:
   1KB-contiguous DMA descriptors (two n-rows per partition).  All
   index-space objects (compat cols, topk idx, selectors, score) live in
   the permuted space n' = j*128 + p; only the dead-mask load needs a
   strided view.  venat f32 is transient: consumed by stage-1 + cast to
   a bf16 copy for stage-3, then freed.
 - Queue split: the sync ring carries ONLY the 1MB venat streams (no
   head-of-line blocking); every small DMA goes on the scalar ring.
 - compat = sum_h ve*t: t broadcast via one K=2 PE matmul per 512 cols
   (hi/lo f32r planes stacked on adjacent partitions, exact f32 sum);
   multiply split gpsimd/DVE; DVE scratch+tbs live in PSUM to relieve
   the shared gpsimd/DVE SBUF port; gpsimd reduces its own j1 share.
 - t packs batched: one [16, 1024] tile per group holds all 8 steps'
   hi/lo planes on partition pairs (1 DMA per group).
 - stage-3 gather: selector is the 17-col STATIONARY, ve_bf16 moving
   (1 cyc/row); 4 ba per PSUM tile via col tile_position; one PE
   transpose per chunk restores [h, (ba,j)].  Gathered values / u are
   bf16 (tol 2e-2); compat/top-k stay f32.
 - heads: W_fwd blocks 1..16 and W_mot@WvT in bf16; vs-terms exact f32.
"""
import math
import os
import sys

for _p in ("/opt/trn_rl_repo", "/root/.axon_site/_ro/trn_rl_repo"):
    if os.path.isdir(_p) and _p not in sys.path:
        sys.path.insert(0, _p)

import numpy as np
import concourse.tile as tile
from concourse import bacc, bass_utils, mybir

F32 = mybir.dt.float32
F32R = mybir.dt.float32r
BF16 = mybir.dt.bfloat16
I32 = mybir.dt.int32
U16 = mybir.dt.uint16
AX = mybir.AxisListType
ALU = mybir.AluOpType
ACTF = mybir.ActivationFunctionType

N_CORES = 8
B, A = 64, 32
N = 256    # entries per (b,a)
H = 128    # head dim
K16 = 16   # top-k
GRP = 64   # (b,a) pairs per processing group
QB = 8     # ba per DMA / pipeline step
NQ = GRP // QB  # steps per group (8)
CH = 4     # ba per stage-3 psum chunk
NCH = GRP // CH  # chunks per group (16)
GB1 = 4    # j1 b-columns multiplied on gpsimd

NEG_MASK = -1.0e30   # added to masked entries
NEG_REPL = -3.0e38   # match_replace fill (below any real/masked value)

_CACHE = {}


def _build(nc, B_pc):
    NBA = 32 * B_pc
    assert NBA % GRP == 0
    NG = NBA // GRP

    ve_d = nc.dram_tensor("ve", [NBA, N, H], F32, kind="ExternalInput")
    vs_d = nc.dram_tensor("vs", [NBA, H], F32, kind="ExternalInput")
    dead_d = nc.dram_tensor("dead", [NBA, N], I32, kind="ExternalInput")
    wq_d = nc.dram_tensor("wq", [H, H], F32, kind="ExternalInput")
    wk_d = nc.dram_tensor("wk", [H, H], F32, kind="ExternalInput")
    wv_d = nc.dram_tensor("wv", [H, H], F32, kind="ExternalInput")
    wmot_d = nc.dram_tensor("wmot", [H, 2 * H], F32, kind="ExternalInput")
    bmot_d = nc.dram_tensor("bmot", [H, 1], F32, kind="ExternalInput")
    wfwd_d = nc.dram_tensor("wfwd", [H, (K16 + 1) * H], F32, kind="ExternalInput")
    bfwd_d = nc.dram_tensor("bfwd", [H, 1], F32, kind="ExternalInput")
    vc_d = nc.dram_tensor("vc", [NBA, H], F32, kind="ExternalOutput")
    vm_d = nc.dram_tensor("vm", [NBA, H], F32, kind="ExternalOutput")

    with tile.TileContext(nc) as tc:
        _body(nc, tc, NBA, NG, ve_d, vs_d, dead_d, wq_d, wk_d, wv_d,
              wmot_d, bmot_d, wfwd_d, bfwd_d, vc_d, vm_d)


def _body(nc, tc, NBA, NG, ve_d, vs_d, dead_d, wq_d, wk_d, wv_d,
          wmot_d, bmot_d, wfwd_d, bfwd_d, vc_d, vm_d):
    from contextlib import ExitStack
    with ExitStack() as ctx:
        consts = ctx.enter_context(tc.tile_pool(name="consts", bufs=1))
        wres = ctx.enter_context(tc.tile_pool(name="wres", bufs=1))
        venat_pool = ctx.enter_context(tc.tile_pool(name="venat", bufs=3))
        vbf_pool = ctx.enter_context(tc.tile_pool(name="vbf", bufs=12))
        scr_pool = ctx.enter_context(tc.tile_pool(name="scr", bufs=2))
        tbs_pool = ctx.enter_context(tc.tile_pool(name="tbs", bufs=2))
        tpk_pool = ctx.enter_context(tc.tile_pool(name="tpk", bufs=2))
        tpre = ctx.enter_context(tc.tile_pool(name="tpre", bufs=2))
        vst_pool = ctx.enter_context(tc.tile_pool(name="vst", bufs=2))
        grp_pool = ctx.enter_context(tc.tile_pool(name="grp", bufs=2))
        grp1 = ctx.enter_context(tc.tile_pool(name="grp1", bufs=2))
        selp = ctx.enter_context(tc.tile_pool(name="selp", bufs=1))
        xsb_pool = ctx.enter_context(tc.tile_pool(name="xsb", bufs=3))
        small = ctx.enter_context(tc.tile_pool(name="small", bufs=3))
        dram_pool = ctx.enter_context(tc.tile_pool(name="dram", bufs=4, space="DRAM"))
        # PSUM budget, bank-granular (8 banks of 2KB/partition):
        #   ps_tbs [128,1024]f32 x2            = 4 banks
        #   ps_tr  [128,256]f32 x1             = 1 bank

        #   ps_x   [128,256]f32 persistent     = 1 bank
        #   ps_xt  [128,256]bf16 persistent    = 1 bank
        ps_tbs = ctx.enter_context(tc.tile_pool(name="ps_tbs", bufs=2, space="PSUM"))
        ps_tr = ctx.enter_context(tc.tile_pool(name="ps_tr", bufs=1, space="PSUM"))
        ps_x = ctx.enter_context(tc.tile_pool(name="ps_x", bufs=1, space="PSUM"))
        ps_xt = ctx.enter_context(tc.tile_pool(name="ps_xt", bufs=1, space="PSUM"))

        # ---- constants ----
        iota_n = consts.tile([128, 128], I32)
        nc.gpsimd.iota(iota_n[:], pattern=[[1, 128]], base=0, channel_multiplier=0)
        iota_p = consts.tile([128, 1], F32)
        nc.gpsimd.iota(iota_p[:], pattern=[[0, 1]], base=0, channel_multiplier=1,
                       allow_small_or_imprecise_dtypes=True)
        iota_p2 = consts.tile([128, 1], F32)  # p + 128
        nc.gpsimd.iota(iota_p2[:], pattern=[[0, 1]], base=128, channel_multiplier=1,
                       allow_small_or_imprecise_dtypes=True)
        ident_f = consts.tile([128, 128], F32)
        nc.vector.tensor_scalar(ident_f[:], iota_n[:], iota_p[:], None,
                                op0=ALU.is_equal)
        ident_bf = consts.tile([128, 128], BF16)
        nc.vector.tensor_copy(ident_bf[:], ident_f[:])
        ident_r = consts.tile([128, 128], F32R)
        nc.scalar.copy(ident_r[:], ident_f[:])
        ones2_f = consts.tile([2, 128], F32)
        nc.gpsimd.memset(ones2_f[:], 1.0)
        ones2_r = consts.tile([2, 128], F32R)
        nc.scalar.copy(ones2_r[:], ones2_f[:])
        ones1_bf = consts.tile([1, 128], BF16)
        nc.gpsimd.memset(ones1_bf[:], 1.0)

        def pe_transpose(dst_sb, src_sb, eng=nc.scalar):
            """dst[f, p] = src[p, f] via PE; dst in SBUF (f32 path)."""
            p_in, f_in = src_sb.shape[0], src_sb.shape[1]
            ps = ps_tr.tile([128, 256], F32, tag="tr")
            out = ps[0:f_in, 0:p_in]
            nc.tensor.transpose(out, src_sb, ident_f[0:p_in, 0:p_in])
            eng.copy(dst_sb, out)

        # ---- early weights: only what tprep needs (wq, wkT) ----
        with tc.tile_pool(name="wtmp0", bufs=1) as wtmp0:
            wq = wres.tile([H, H], F32)
            nc.scalar.dma_start(wq[:], wq_d.ap())
            wk = wtmp0.tile([H, H], F32)
            nc.scalar.dma_start(wk[:], wk_d.ap())
            wkT = wres.tile([H, H], F32)
            pe_transpose(wkT[:], wk[:])
        bmot = wres.tile([H, 1], F32)
        nc.scalar.dma_start(bmot[:], bmot_d.ap())
        bfwd = wres.tile([H, 1], F32)
        nc.scalar.dma_start(bfwd[:], bfwd_d.ap())
        wm0T = wres.tile([H, H], F32)
        wmv_bf = wres.tile([H, H], BF16)
        wf0T = wres.tile([H, H], F32)
        wf_bf = wres.tile([H, K16 * H], BF16)

        def emit_late_weights():
            """head weights: emitted after stage1(0) so the transposes
            overlap the streaming pipeline instead of delaying it."""
            with tc.tile_pool(name="wtmp", bufs=1) as wtmp:
                wv = wtmp.tile([H, H], F32)
                nc.scalar.dma_start(wv[:], wv_d.ap())
                wmot = wtmp.tile([H, 2 * H], F32)
                nc.scalar.dma_start(wmot[:], wmot_d.ap())
                wfwd = wtmp.tile([H, (K16 + 1) * H], F32)
                nc.scalar.dma_start(wfwd[:], wfwd_d.ap())
                wvT = wtmp.tile([H, H], F32)
                pe_transpose(wvT[:], wv[:])
                pe_transpose(wm0T[:], wmot[:, 0:H])
                wm1T = wtmp.tile([H, H], F32)
                pe_transpose(wm1T[:], wmot[:, H:2 * H])
                wmvT_f = wtmp.tile([H, H], F32)
                ps = ps_tr.tile([128, 256], F32, tag="tr")
                nc.tensor.matmul(ps[:, 0:128], wvT[:], wm1T[:])
                nc.scalar.copy(wmvT_f[:], ps[:, 0:128])
                nc.scalar.copy(wmv_bf[:], wmvT_f[:])
                pe_transpose(wf0T[:], wfwd[:, 0:H])
                for j in range(1, K16 + 1):
                    pe_transpose(wf_bf[:, (j - 1) * H:j * H],
                                 wfwd[:, j * H:(j + 1) * H])

        # ---- per-group state ----
        xps_all = ps_x.tile([128, 256], F32, tag="x")
        vpair_box = [None]
        xt_all = ps_xt.tile([128, 256], BF16, tag="xt")
        vst_f = {}       # g -> vs^T tile [H, GRP] f32
        tpk16_g = {}     # g -> [16, QB*H] f32r (hi/lo planes per step)
        vbf_g = {}       # (g, t8) -> bf16 venat tile [128, QB*N]
        cc_g = {}        # g -> cc tile [128, 2*GRP] ([p, (j, ba)])
        sel_g = {}       # g -> (s_a, s_b) bf16 [128, GRP*17]
        xq_g = {}        # g -> gathered tile [128, NCH*128] bf16

        def emit_tprep(g):
            """t = (Wk^T Wq^T vs)/sqrt(H) rows, split hi/lo f32r, batched
            into one [16, QB*H] pack per group via a DRAM bounce."""
            vs_rows = tpre.tile([GRP, H], F32, tag="vsrows")
            nc.scalar.dma_start(vs_rows[:], vs_d.ap()[g * GRP:(g + 1) * GRP, :])
            vstf = vst_pool.tile([H, GRP], F32, tag="vstf")
            pe_transpose(vstf[:], vs_rows[:])
            qt = tpre.tile([H, GRP], F32, tag="qt")
            ps = ps_tr.tile([128, 256], F32, tag="tr")
            nc.tensor.matmul(ps[:, 0:GRP], wq[:], vstf[:])
            nc.scalar.copy(qt[:], ps[:, 0:GRP])
            tsb = tpre.tile([H, GRP], F32, tag="tsb")
            ps = ps_tr.tile([128, 256], F32, tag="tr")
            nc.tensor.matmul(ps[:, 0:GRP], wkT[:], qt[:])
            nc.scalar.mul(tsb[:], ps[:, 0:GRP], 1.0 / math.sqrt(H))
            trows_f = tpre.tile([GRP, H], F32, tag="trowsf")
            pe_transpose(trows_f[:], tsb[:])
            trows_r = tpre.tile([GRP, H], F32R, tag="trowsr")
            nc.scalar.copy(trows_r[:], trows_f[:])
            tlo_r = tpre.tile([GRP, H], F32R, tag="tlor")
            nc.vector.tensor_tensor(tlo_r[:], trows_f[:], trows_r[:].bitcast(F32),
                                    op=ALU.subtract)
            t_dram = dram_pool.tile([2, GRP, H], F32R, tag="tdram")
            nc.scalar.dma_start(t_dram[:][0], trows_r[:])
            nc.scalar.dma_start(t_dram[:][1], tlo_r[:])
            # packs of 2 steps: [2 planes, (q2, b, h)]
            packs = []
            for half in range(NQ // 2):
                tpk4 = tpk_pool.tile([2, 2 * QB * H], F32R, tag="tpk4")
                nc.scalar.dma_start(
                    tpk4[:].rearrange("pl (q b h) -> pl q b h", q=2, b=QB),
                    t_dram[:].rearrange("pl (q b) h -> pl q b h", b=QB)
                    [:, half * 2:(half + 1) * 2])
                packs.append(tpk4)
            vst_f[g] = vstf
            tpk16_g[g] = packs

        def emit_stage1_qb(g, q):
            """load QB ba's of ve (parity layout), broadcast t,
            multiply+reduce, cast bf16."""
            ib = g * GRP + q * QB
            if q == 0:
                cc_g[g] = grp_pool.tile([128, 2 * GRP], F32, tag="cc", name="cc")
            cc = cc_g[g]
            if q % 2 == 0:
                vpair = venat_pool.tile([128, 2 * QB * N], F32, tag="venat")
                src = ve_d.ap()[ib:ib + 2 * QB].rearrange(
                    "b (p j) h -> p b j h", j=2)
                nc.sync.dma_start(
                    vpair[:].rearrange("p (b j h) -> p b j h", b=2 * QB, j=2),
                    src)
                vpair_box[0] = vpair
            venat = vpair_box[0][:][:, (q % 2) * QB * N:(q % 2 + 1) * QB * N]
            # broadcast t across partitions: K=2 matmul sums hi+lo exactly
            tpk4 = tpk16_g[g][q // 2]
            qo = (q % 2) * QB * H
            tbs_ps = ps_tbs.tile([128, QB * H], F32, tag="tbs")
            nc.tensor.matmul(tbs_ps[:, 0:512],
                             ones2_r[:], tpk4[:, qo:qo + 512],
                             start=True, stop=True)
            nc.tensor.matmul(tbs_ps[:, 512:1024],
                             ones2_r[:], tpk4[:, qo + 512:qo + 1024],
                             start=True, stop=True)
            tbs = tbs_pool.tile([128, QB * H], F32, tag="tbs")
            nc.scalar.copy(tbs[:], tbs_ps[:])
            vfull = venat.rearrange("p (b j h) -> p b j h", b=QB, j=2)
            tb = tbs[:].rearrange("p (b h) -> p b h", b=QB)
            tbp = tbs_ps[:].rearrange("p (b h) -> p b h", b=QB)
            # gpsimd: j0 all b + j1 b[0:GB1] (mult+reduce); DVE: j1 b[GB1:]
            # with PSUM tbs + PSUM scratch (keeps the shared SBUF port free)
            scr = scr_pool.tile([128, 2 * QB * H], F32, tag="scr")
            s0 = scr[:].rearrange("p (b h) -> p b h", b=2 * QB)
            nc.gpsimd.tensor_tensor(s0[:, 0:QB, :], vfull[:, :, 0, :], tb,
                                    op=ALU.mult)
            nc.gpsimd.tensor_tensor(s0[:, QB:QB + GB1, :], vfull[:, 0:GB1, 1, :],
                                    tb[:, 0:GB1, :], op=ALU.mult)
            nc.vector.tensor_tensor(s0[:, QB + GB1:2 * QB, :],
                                    vfull[:, GB1:QB, 1, :],
                                    tbp[:, GB1:QB, :], op=ALU.mult)
            nc.vector.tensor_reduce(cc[:, q * QB:(q + 1) * QB], s0[:, 0:QB, :],
                                    axis=AX.X, op=ALU.add)
            nc.vector.tensor_reduce(
                cc[:, GRP + q * QB: GRP + (q + 1) * QB],
                s0[:, QB:2 * QB, :], axis=AX.X, op=ALU.add)
            # bf16 copy for stage-3 (gather + u)
            vbf = vbf_pool.tile([128, QB * N], BF16, tag="vbf")
            nc.scalar.copy(vbf[:], venat)
            vbf_g[(g, q)] = vbf

        def emit_stage2(g):
            """softmax + top-16 + bf16 selector build for group g.
            All index-space objects live in n' = j*128 + p order."""
            cc = cc_g[g]
            cmp_ps = ps_tr.tile([128, 256], F32, tag="tr")
            nc.tensor.transpose(cmp_ps[0:GRP, 0:128], cc[:, 0:GRP],
                                ident_f[:])
            nc.tensor.transpose(cmp_ps[0:GRP, 128:256], cc[:, GRP:2 * GRP],
                                ident_f[:])

            dead_i = grp1.tile([GRP, N], I32, tag="deadi")
            nc.sync.dma_start(dead_i[:], dead_d.ap()[g * GRP:(g + 1) * GRP, :])
            dead_f = grp1.tile([GRP, N], F32, tag="deadf")
            nc.vector.tensor_copy(dead_f[:], dead_i[:])
            cm_sb = grp1.tile([GRP, N], F32, tag="cmsb")
            # dead is in raw n order; view it in n' = (j, p) order
            nc.vector.scalar_tensor_tensor(
                cm_sb[:].rearrange("g (j p) -> g j p", j=2),
                dead_f[:].rearrange("g (p j) -> g j p", j=2),
                NEG_MASK,
                cmp_ps[0:GRP, :].rearrange("g (j p) -> g j p", j=2),
                op0=ALU.mult, op1=ALU.add)

            mx_neg = small.tile([GRP, 1], F32, tag="mxneg")
            nc.vector.tensor_reduce(mx_neg[:], cm_sb[:], axis=AX.X, op=ALU.max,
                                    negate=True)
            score_un = grp1.tile([GRP, N], F32, tag="scoreun")
            ssum = small.tile([GRP, 1], F32, tag="ssum")
            nc.scalar.activation(score_un[:], cm_sb[:], ACTF.Exp,
                                 bias=mx_neg[:], scale=1.0, accum_out=ssum[:])
            rs = small.tile([GRP, 1], F32, tag="rs")
            nc.vector.reciprocal(rs[:], ssum[:])
            score_bf = grp1.tile([GRP, N], BF16, tag="scorebf")
            nc.vector.tensor_scalar_mul(score_bf[:], score_un[:], rs[:])

            # top-16 (two rounds of max8 + find_index8), idx in n' space
            mx8a = small.tile([GRP, 8], F32, tag="mx8a")
            nc.vector.max(mx8a[:], cm_sb[:])
            idx16 = small.tile([GRP, K16], U16, tag="idx16")
            nc.vector.max_index(idx16[:, 0:8], mx8a[:], cm_sb[:])
            cm2 = grp1.tile([GRP, N], F32, tag="cm2")
            nc.vector.match_replace(cm2[:], mx8a[:], cm_sb[:], NEG_REPL)
            mx8b = small.tile([GRP, 8], F32, tag="mx8b")
            nc.vector.max(mx8b[:], cm2[:])
            nc.vector.max_index(idx16[:, 8:16], mx8b[:], cm2[:])
            idx_bf = small.tile([GRP, K16], BF16, tag="idxbf")
            nc.vector.tensor_copy(idx_bf[:], idx16[:])
            # flatten idx rows onto one partition via SBUF->SBUF DMA
            idx_pack = tpk_pool.tile([1, GRP * K16], BF16, tag="idxpack")
            nc.sync.dma_start(
                idx_pack[:].rearrange("p (b k) -> p b k", k=K16), idx_bf[:])
            # broadcast indices to all partitions: [128, (ba, j)]
            idx_ps = ps_tbs.tile([128, 1024], F32, tag="tbs")
            nc.tensor.matmul(idx_ps[:, 0:512], ones1_bf[:], idx_pack[:, 0:512],
                             start=True, stop=True)
            nc.tensor.matmul(idx_ps[:, 512:1024], ones1_bf[:],
                             idx_pack[:, 512:1024], start=True, stop=True)
            idx_sb = tbs_pool.tile([128, GRP * K16], BF16, tag="idxsb")
            nc.scalar.copy(idx_sb[:], idx_ps[:])
            # selectors: s[p, ba, j] = (idx[ba, j] == n'(p)) ; col 17 = score
            s_a = selp.tile([128, GRP * (K16 + 1)], BF16, tag="sa")
            s_b = selp.tile([128, GRP * (K16 + 1)], BF16, tag="sb")
            s_a_v = s_a[:].rearrange("p (b j) -> p b j", j=K16 + 1)
            s_b_v = s_b[:].rearrange("p (b j) -> p b j", j=K16 + 1)
            idx_v = idx_sb[:].rearrange("p (b j) -> p b j", j=K16)
            nc.vector.tensor_scalar(s_a_v[:, :, 0:K16], idx_v, iota_p[:], None,
                                    op0=ALU.is_equal)
            nc.vector.tensor_scalar(s_b_v[:, :, 0:K16], idx_v, iota_p2[:], None,
                                    op0=ALU.is_equal)
            # score columns: transpose [ba, n'] -> [n', ba] (bf16)
            st_f = ps_tr.tile([128, 256], F32, tag="tr")
            st_ps = st_f[:].bitcast(BF16)[:, 0:256]
            nc.tensor.transpose(st_ps[0:128, 0:GRP], score_bf[:, 0:128],
                                ident_bf[0:GRP, 0:GRP])
            nc.tensor.transpose(st_ps[0:128, GRP:2 * GRP], score_bf[:, 128:256],
                                ident_bf[0:GRP, 0:GRP])
            nc.scalar.copy(s_a_v[:, :, K16], st_ps[0:128, 0:GRP])
            nc.scalar.copy(s_b_v[:, :, K16], st_ps[0:128, GRP:2 * GRP])
            sel_g[g] = (s_a, s_b)
            xq_g[g] = grp_pool.tile([128, NCH * 128], BF16, tag="xq", name="xq")

        def emit_stage3_chunk(g, c):
            """gather+u for ba in [c*CH, (c+1)*CH): sel-stationary bf16 MMs,
            4 ba packed per psum tile via col tile_position, one PE
            transpose restores [h, (ba-chunk cols)]."""
            s_a, s_b = sel_g[g]
            par = c % 2
            xps = xps_all[:][:, par * 128:(par + 1) * 128]
            for phase in range(2):
                for q4 in range(CH):
                    ba = c * CH + q4
                    vb = vbf_g[(g, ba // QB)]
                    base = (ba % QB) * N
                    lo, hi = ba * 17, (ba + 1) * 17
                    if phase == 0:
                        nc.tensor.matmul(xps[32 * q4:32 * q4 + 17, :],
                                         s_a[:, lo:hi], vb[:, base:base + 128],
                                         start=True, stop=False,
                                         tile_position=(0, 32 * q4))
                    else:
                        nc.tensor.matmul(xps[32 * q4:32 * q4 + 17, :],
                                         s_b[:, lo:hi],
                                         vb[:, base + 128:base + 256],
                                         start=False, stop=True,
                                         tile_position=(0, 32 * q4))
            x_sb = xsb_pool.tile([128, 128], BF16, tag="xsb")
            nc.scalar.copy(x_sb[:], xps)
            xt_ps = xt_all[:][:, par * 128:(par + 1) * 128]
            nc.tensor.transpose(xt_ps, x_sb[:], ident_bf[:])
            nc.vector.tensor_copy(xq_g[g][:, c * 128:(c + 1) * 128], xt_ps)

        def emit_heads(g):
            """vC / vM heads for group g. xq col = c*128 + 32*q4 + j."""
            xq = xq_g[g]
            xq_v = xq[:].rearrange("p (c q w) -> p c q w", q=CH, w=32)
            vc_ps = ps_tr.tile([128, 256], F32, tag="tr")
            nc.tensor.matmul(vc_ps[:, 0:GRP], wf0T[:], vst_f[g][:],
                             start=True, stop=False)
            for j in range(1, K16 + 1):
                nc.tensor.matmul(vc_ps[:, 0:GRP],
                                 wf_bf[:, (j - 1) * H:j * H],
                                 xq_v[:, :, :, j - 1],
                                 start=False, stop=(j == K16))
            vc_sb = grp1.tile([128, GRP], F32, tag="vcsb")
            nc.scalar.activation(vc_sb[:], vc_ps[:, 0:GRP], ACTF.Relu,
                                 bias=bfwd[:], scale=1.0)
            vc_rows = grp1.tile([GRP, H], F32, tag="vcrows")
            pe_transpose(vc_rows[:], vc_sb[:])
            nc.scalar.dma_start(vc_d.ap()[g * GRP:(g + 1) * GRP, :], vc_rows[:])

            vm_ps = ps_tr.tile([128, 256], F32, tag="tr")
            nc.tensor.matmul(vm_ps[:, 0:GRP], wm0T[:], vst_f[g][:],
                             start=True, stop=False)
            nc.tensor.matmul(vm_ps[:, 0:GRP], wmv_bf[:], xq_v[:, :, :, K16],
                             start=False, stop=True)
            vm_sb = grp1.tile([128, GRP], F32, tag="vmsb")
            nc.scalar.activation(vm_sb[:], vm_ps[:, 0:GRP], ACTF.Relu,
                                 bias=bmot[:], scale=1.0)
            vm_rows = grp1.tile([GRP, H], F32, tag="vmrows")
            pe_transpose(vm_rows[:], vm_sb[:])
            nc.scalar.dma_start(vm_d.ap()[g * GRP:(g + 1) * GRP, :], vm_rows[:])

        # ---- software-pipelined emission ----
        emit_tprep(0)
        for q in range(NQ):
            emit_stage1_qb(0, q)
        emit_late_weights()
        for g in range(NG):
            emit_stage2(g)
            if g + 1 < NG:
                emit_tprep(g + 1)
            for q in range(NQ):
                emit_stage3_chunk(g, 2 * q)
                emit_stage3_chunk(g, 2 * q + 1)
                if g + 1 < NG:
                    emit_stage1_qb(g + 1, q)
            emit_heads(g)
            for q in range(NQ):
                del vbf_g[(g, q)]


def _get_compiled(B_pc):
    key = B_pc
    if key not in _CACHE:
        nc = bacc.Bacc("TRN2", target_bir_lowering=False, debug=False,
                       num_devices=N_CORES)
        _build(nc, B_pc)
        nc.compile()
        _CACHE[key] = nc
    return _CACHE[key]


def kernel(vs, ve, ve_dead, Wq, Wk, Wv, W_mot, b_mot, W_fwd, b_fwd,
           trace=False, trace_kwargs=None):
    vs = np.asarray(vs, dtype=np.float32)
    ve = np.asarray(ve, dtype=np.float32)
    ve_dead = np.asarray(ve_dead, dtype=np.int32)
    Bq, Aq = vs.shape[0], vs.shape[1]
    assert (Bq, Aq) == (B, A), (Bq, Aq)
    B_pc = B // N_CORES
    NBA = B_pc * A

    nc = _get_compiled(B_pc)

    shared = {
        "wq": np.ascontiguousarray(Wq, dtype=np.float32),
        "wk": np.ascontiguousarray(Wk, dtype=np.float32),
        "wv": np.ascontiguousarray(Wv, dtype=np.float32),
        "wmot": np.ascontiguousarray(W_mot, dtype=np.float32),
        "bmot": np.ascontiguousarray(b_mot, dtype=np.float32).reshape(H, 1),
        "wfwd": np.ascontiguousarray(W_fwd, dtype=np.float32),
        "bfwd": np.ascontiguousarray(b_fwd, dtype=np.float32).reshape(H, 1),
    }
    in_maps = []
    for c in range(N_CORES):
        sl = slice(c * B_pc, (c + 1) * B_pc)
        in_maps.append({
            "ve": np.ascontiguousarray(ve[sl].reshape(NBA, N, H)),
            "vs": np.ascontiguousarray(vs[sl].reshape(NBA, H)),
            "dead": np.ascontiguousarray(ve_dead[sl].reshape(NBA, N)),
            **shared,
        })

    res = bass_utils.run_bass_kernel_spmd(
        nc, in_maps, core_ids=list(range(N_CORES)),
        trace=trace, **(trace_kwargs or {}))

    vc = np.empty((B, A, H), dtype=np.float32)
    vm = np.empty((B, A, H), dtype=np.float32)
    for c in range(N_CORES):
        sl = slice(c * B_pc, (c + 1) * B_pc)
        vc[sl] = res.results[c]["vc"].reshape(B_pc, A, H)
        vm[sl] = res.results[c]["vm"].reshape(B_pc, A, H)
    kernel.last_results = res
    return (vc, vm)


# revision 23
# speedup vs baseline: 1.0029x; 1.0029x over previous
"""Trainium2 Bass kernel for nn_Concentration_61229053772314.

kernel(**inputs) takes the FULL inputs (B=64), shards the batch dim across
8 NeuronCores (pure data parallel, weights replicated), runs a Bass/Tile
kernel via run_bass_kernel_spmd, and reassembles the full outputs.

v5 architecture (per core: NBA=256 (b,a) rows, 4 groups of GRP=64):
 - ve streamed once as f32 in parity layout [p, (b, j, h)] with n = 2p+j:
   1KB-contiguous DMA descriptors (two n-rows per partition).  All
   index-space objects (compat cols, topk idx, selectors, score) live in
   the permuted space n' = j*128 + p; only the dead-mask load needs a
   strided view.  venat f32 is transient: consumed by stage-1 + cast to
   a bf16 copy for stage-3, then freed.
 - Queue split: the sync ring carries ONLY the 1MB venat streams (no
   head-of-line blocking); every small DMA goes on the scalar ring.
 - compat = sum_h ve*t: t broadcast via one K=2 PE matmul per 512 cols
   (hi/lo f32r planes stacked on adjacent partitions, exact f32 sum);
   multiply split gpsimd/DVE; DVE scratch+tbs live in PSUM to relieve
   the shared gpsimd/DVE SBUF port; gpsimd reduces its own j1 share.
 - t packs batched: one [16, 1024] tile per group holds all 8 steps'
   hi/lo planes on partition pairs (1 DMA per group).
 - stage-3 gather: selector is the 17-col STATIONARY, ve_bf16 moving
   (1 cyc/row); 4 ba per PSUM tile via col tile_position; one PE
   transpose per chunk restores [h, (ba,j)].  Gathered values / u are
   bf16 (tol 2e-2); compat/top-k stay f32.
 - heads: W_fwd blocks 1..16 and W_mot@WvT in bf16; vs-terms exact f32.
"""
import math
import os
import sys

for _p in ("/opt/trn_rl_repo", "/root/.axon_site/_ro/trn_rl_repo"):
    if os.path.isdir(_p) and _p not in sys.path:
        sys.path.insert(0, _p)

import numpy as np
import concourse.tile as tile
from concourse import bacc, bass_utils, mybir

F32 = mybir.dt.float32
F32R = mybir.dt.float32r
BF16 = mybir.dt.bfloat16
I32 = mybir.dt.int32
U16 = mybir.dt.uint16
AX = mybir.AxisListType
ALU = mybir.AluOpType
ACTF = mybir.ActivationFunctionType

N_CORES = 8
B, A = 64, 32
N = 256    # entries per (b,a)
H = 128    # head dim
K16 = 16   # top-k
GRP = 64   # (b,a) pairs per processing group
QB = 8     # ba per DMA / pipeline step
NQ = GRP // QB  # steps per group (8)
CH = 4     # ba per stage-3 psum chunk
NCH = GRP // CH  # chunks per group (16)
GB1 = 3    # j1 b-columns multiplied on gpsimd

NEG_MASK = -1.0e30   # added to masked entries
NEG_REPL = -3.0e38   # match_replace fill (below any real/masked value)

_CACHE = {}


def _build(nc, B_pc):
    NBA = 32 * B_pc
    assert NBA % GRP == 0
    NG = NBA // GRP

    ve_d = nc.dram_tensor("ve", [NBA, N, H], F32, kind="ExternalInput")
    vs_d = nc.dram_tensor("vs", [NBA, H], F32, kind="ExternalInput")
    dead_d = nc.dram_tensor("dead", [NBA, N], I32, kind="ExternalInput")
    wq_d = nc.dram_tensor("wq", [H, H], F32, kind="ExternalInput")
    wk_d = nc.dram_tensor("wk", [H, H], F32, kind="ExternalInput")
    wv_d = nc.dram_tensor("wv", [H, H], F32, kind="ExternalInput")
    wmot_d = nc.dram_tensor("wmot", [H, 2 * H], F32, kind="ExternalInput")
    bmot_d = nc.dram_tensor("bmot", [H, 1], F32, kind="ExternalInput")
    wfwd_d = nc.dram_tensor("wfwd", [H, (K16 + 1) * H], F32, kind="ExternalInput")
    bfwd_d = nc.dram_tensor("bfwd", [H, 1], F32, kind="ExternalInput")
    vc_d = nc.dram_tensor("vc", [NBA, H], F32, kind="ExternalOutput")
    vm_d = nc.dram_tensor("vm", [NBA, H], F32, kind="ExternalOutput")

    with tile.TileContext(nc) as tc:
        _body(nc, tc, NBA, NG, ve_d, vs_d, dead_d, wq_d, wk_d, wv_d,
              wmot_d, bmot_d, wfwd_d, bfwd_d, vc_d, vm_d)


def _body(nc, tc, NBA, NG, ve_d, vs_d, dead_d, wq_d, wk_d, wv_d,
          wmot_d, bmot_d, wfwd_d, bfwd_d, vc_d, vm_d):
    from contextlib import ExitStack
    with ExitStack() as ctx:
        consts = ctx.enter_context(tc.tile_pool(name="consts", bufs=1))
        wres = ctx.enter_context(tc.tile_pool(name="wres", bufs=1))
        venat_pool = ctx.enter_context(tc.tile_pool(name="venat", bufs=2))
        vbf_pool = ctx.enter_context(tc.tile_pool(name="vbf", bufs=10))
        scr_pool = ctx.enter_context(tc.tile_pool(name="scr", bufs=2))
        tbs_pool = ctx.enter_context(tc.tile_pool(name="tbs", bufs=2))
        tpk_pool = ctx.enter_context(tc.tile_pool(name="tpk", bufs=2))
        tpre = ctx.enter_context(tc.tile_pool(name="tpre", bufs=2))
        vst_pool = ctx.enter_context(tc.tile_pool(name="vst", bufs=2))
        grp_pool = ctx.enter_context(tc.tile_pool(name="grp", bufs=2))
        grp1 = ctx.enter_context(tc.tile_pool(name="grp1", bufs=2))
        selp = ctx.enter_context(tc.tile_pool(name="selp", bufs=1))
        xsb_pool = ctx.enter_context(tc.tile_pool(name="xsb", bufs=3))
        small = ctx.enter_context(tc.tile_pool(name="small", bufs=3))
        dram_pool = ctx.enter_context(tc.tile_pool(name="dram", bufs=4, space="DRAM"))
        # PSUM budget, bank-granular (8 banks of 2KB/partition):
        #   ps_tbs [128,1024]f32 x2            = 4 banks
        #   ps_tr  [128,256]f32 x1             = 1 bank

        #   ps_x   [128,256]f32 persistent     = 1 bank
        #   ps_xt  [128,256]bf16 persistent    = 1 bank
        ps_tbs = ctx.enter_context(tc.tile_pool(name="ps_tbs", bufs=2, space="PSUM"))
        ps_tr = ctx.enter_context(tc.tile_pool(name="ps_tr", bufs=1, space="PSUM"))
        ps_x = ctx.enter_context(tc.tile_pool(name="ps_x", bufs=1, space="PSUM"))
        ps_xt = ctx.enter_context(tc.tile_pool(name="ps_xt", bufs=1, space="PSUM"))

        # ---- constants ----
        iota_n = consts.tile([128, 128], I32)
        nc.gpsimd.iota(iota_n[:], pattern=[[1, 128]], base=0, channel_multiplier=0)
        iota_p = consts.tile([128, 1], F32)
        nc.gpsimd.iota(iota_p[:], pattern=[[0, 1]], base=0, channel_multiplier=1,
                       allow_small_or_imprecise_dtypes=True)
        iota_p2 = consts.tile([128, 1], F32)  # p + 128
        nc.gpsimd.iota(iota_p2[:], pattern=[[0, 1]], base=128, channel_multiplier=1,
                       allow_small_or_imprecise_dtypes=True)
        ident_f = consts.tile([128, 128], F32)
        nc.vector.tensor_scalar(ident_f[:], iota_n[:], iota_p[:], None,
                                op0=ALU.is_equal)
        ident_bf = consts.tile([128, 128], BF16)
        nc.vector.tensor_copy(ident_bf[:], ident_f[:])
        ident_r = consts.tile([128, 128], F32R)
        nc.scalar.copy(ident_r[:], ident_f[:])
        ones2_f = consts.tile([2, 128], F32)
        nc.gpsimd.memset(ones2_f[:], 1.0)
        ones2_r = consts.tile([2, 128], F32R)
        nc.scalar.copy(ones2_r[:], ones2_f[:])
        ones1_bf = consts.tile([1, 128], BF16)
        nc.gpsimd.memset(ones1_bf[:], 1.0)

        def pe_transpose(dst_sb, src_sb, eng=nc.scalar):
            """dst[f, p] = src[p, f] via PE; dst in SBUF (f32 path)."""
            p_in, f_in = src_sb.shape[0], src_sb.shape[1]
            ps = ps_tr.tile([128, 256], F32, tag="tr")
            out = ps[0:f_in, 0:p_in]
            nc.tensor.transpose(out, src_sb, ident_f[0:p_in, 0:p_in])
            eng.copy(dst_sb, out)

        # ---- early weights: only what tprep needs (wq, wkT) ----
        with tc.tile_pool(name="wtmp0", bufs=1) as wtmp0:
            wq = wres.tile([H, H], F32)
            nc.scalar.dma_start(wq[:], wq_d.ap())
            wk = wtmp0.tile([H, H], F32)
            nc.scalar.dma_start(wk[:], wk_d.ap())
            wkT = wres.tile([H, H], F32)
            pe_transpose(wkT[:], wk[:])
        bmot = wres.tile([H, 1], F32)
        nc.scalar.dma_start(bmot[:], bmot_d.ap())
        bfwd = wres.tile([H, 1], F32)
        nc.scalar.dma_start(bfwd[:], bfwd_d.ap())
        wm0T = wres.tile([H, H], F32)
        wmv_bf = wres.tile([H, H], BF16)
        wf0T = wres.tile([H, H], F32)
        wf_bf = wres.tile([H, K16 * H], BF16)

        def emit_late_weights():
            """head weights: emitted after stage1(0) so the transposes
            overlap the streaming pipeline instead of delaying it."""
            with tc.tile_pool(name="wtmp", bufs=1) as wtmp:
                wv = wtmp.tile([H, H], F32)
                nc.scalar.dma_start(wv[:], wv_d.ap())
                wmot = wtmp.tile([H, 2 * H], F32)
                nc.scalar.dma_start(wmot[:], wmot_d.ap())
                wfwd = wtmp.tile([H, (K16 + 1) * H], F32)
                nc.scalar.dma_start(wfwd[:], wfwd_d.ap())
                wvT = wtmp.tile([H, H], F32)
                pe_transpose(wvT[:], wv[:])
                pe_transpose(wm0T[:], wmot[:, 0:H])
                wm1T = wtmp.tile([H, H], F32)
                pe_transpose(wm1T[:], wmot[:, H:2 * H])
                wmvT_f = wtmp.tile([H, H], F32)
                ps = ps_tr.tile([128, 256], F32, tag="tr")
                nc.tensor.matmul(ps[:, 0:128], wvT[:], wm1T[:])
                nc.scalar.copy(wmvT_f[:], ps[:, 0:128])
                nc.scalar.copy(wmv_bf[:], wmvT_f[:])
                pe_transpose(wf0T[:], wfwd[:, 0:H])
                for j in range(1, K16 + 1):
                    pe_transpose(wf_bf[:, (j - 1) * H:j * H],
                                 wfwd[:, j * H:(j + 1) * H])

        # ---- per-group state ----
        xps_all = ps_x.tile([128, 256], F32, tag="x")
        vpair_box = [None]
        xt_all = ps_xt.tile([128, 256], BF16, tag="xt")
        vst_f = {}       # g -> vs^T tile [H, GRP] f32
        tpk16_g = {}     # g -> [16, QB*H] f32r (hi/lo planes per step)
        vbf_g = {}       # (g, t8) -> bf16 venat tile [128, QB*N]
        cc_g = {}        # g -> cc tile [128, 2*GRP] ([p, (j, ba)])
        sel_g = {}       # g -> (s_a, s_b) bf16 [128, GRP*17]
        xq_g = {}        # g -> gathered tile [128, NCH*128] bf16

        def emit_tprep(g):
            """t = (Wk^T Wq^T vs)/sqrt(H) rows, split hi/lo f32r, batched
            into one [16, QB*H] pack per group via a DRAM bounce."""
            vs_rows = tpre.tile([GRP, H], F32, tag="vsrows")
            nc.scalar.dma_start(vs_rows[:], vs_d.ap()[g * GRP:(g + 1) * GRP, :])
            vstf = vst_pool.tile([H, GRP], F32, tag="vstf")
            pe_transpose(vstf[:], vs_rows[:])
            qt = tpre.tile([H, GRP], F32, tag="qt")
            ps = ps_tr.tile([128, 256], F32, tag="tr")
            nc.tensor.matmul(ps[:, 0:GRP], wq[:], vstf[:])
            nc.scalar.copy(qt[:], ps[:, 0:GRP])
            tsb = tpre.tile([H, GRP], F32, tag="tsb")
            ps = ps_tr.tile([128, 256], F32, tag="tr")
            nc.tensor.matmul(ps[:, 0:GRP], wkT[:], qt[:])
            nc.scalar.mul(tsb[:], ps[:, 0:GRP], 1.0 / math.sqrt(H))
            trows_f = tpre.tile([GRP, H], F32, tag="trowsf")
            pe_transpose(trows_f[:], tsb[:])
            trows_r = tpre.tile([GRP, H], F32R, tag="trowsr")
            nc.scalar.copy(trows_r[:], trows_f[:])
            tlo_r = tpre.tile([GRP, H], F32R, tag="tlor")
            nc.vector.tensor_tensor(tlo_r[:], trows_f[:], trows_r[:].bitcast(F32),
                                    op=ALU.subtract)
            t_dram = dram_pool.tile([2, GRP, H], F32R, tag="tdram")
            nc.scalar.dma_start(t_dram[:][0], trows_r[:])
            nc.scalar.dma_start(t_dram[:][1], tlo_r[:])
            # packs of 2 steps: [2 planes, (q2, b, h)]
            packs = []
            for half in range(NQ // 2):
                tpk4 = tpk_pool.tile([2, 2 * QB * H], F32R, tag="tpk4")
                nc.scalar.dma_start(
                    tpk4[:].rearrange("pl (q b h) -> pl q b h", q=2, b=QB),
                    t_dram[:].rearrange("pl (q b) h -> pl q b h", b=QB)
                    [:, half * 2:(half + 1) * 2])
                packs.append(tpk4)
            vst_f[g] = vstf
            tpk16_g[g] = packs

        def emit_stage1_qb(g, q):
            """load QB ba's of ve (parity layout), broadcast t,
            multiply+reduce, cast bf16."""
            ib = g * GRP + q * QB
            if q == 0:
                cc_g[g] = grp_pool.tile([128, 2 * GRP], F32, tag="cc", name="cc")
            cc = cc_g[g]
            if q % 4 == 0:
                vpair = venat_pool.tile([128, 4 * QB * N], F32, tag="venat")
                src = ve_d.ap()[ib:ib + 4 * QB].rearrange(
                    "b (p j) h -> p b j h", j=2)
                nc.sync.dma_start(
                    vpair[:].rearrange("p (b j h) -> p b j h", b=4 * QB, j=2),
                    src)
                vpair_box[0] = vpair
            venat = vpair_box[0][:][:, (q % 4) * QB * N:(q % 4 + 1) * QB * N]
            # broadcast t across partitions: K=2 matmul sums hi+lo exactly
            tpk4 = tpk16_g[g][q // 2]
            qo = (q % 2) * QB * H
            tbs_ps = ps_tbs.tile([128, QB * H], F32, tag="tbs")
            nc.tensor.matmul(tbs_ps[:, 0:512],
                             ones2_r[:], tpk4[:, qo:qo + 512],
                             start=True, stop=True)
            nc.tensor.matmul(tbs_ps[:, 512:1024],
                             ones2_r[:], tpk4[:, qo + 512:qo + 1024],
                             start=True, stop=True)
            tbs = tbs_pool.tile([128, QB * H], F32, tag="tbs")
            nc.scalar.copy(tbs[:], tbs_ps[:])
            vfull = venat.rearrange("p (b j h) -> p b j h", b=QB, j=2)
            tb = tbs[:].rearrange("p (b h) -> p b h", b=QB)
            tbp = tbs_ps[:].rearrange("p (b h) -> p b h", b=QB)
            # gpsimd: j0 all b + j1 b[0:GB1] (mult+reduce); DVE: j1 b[GB1:]
            # with PSUM tbs + PSUM scratch (keeps the shared SBUF port free)
            scr = scr_pool.tile([128, 2 * QB * H], F32, tag="scr")
            s0 = scr[:].rearrange("p (b h) -> p b h", b=2 * QB)
            nc.gpsimd.tensor_tensor(s0[:, 0:QB, :], vfull[:, :, 0, :], tb,
                                    op=ALU.mult)
            nc.gpsimd.tensor_tensor(s0[:, QB:QB + GB1, :], vfull[:, 0:GB1, 1, :],
                                    tb[:, 0:GB1, :], op=ALU.mult)
            nc.vector.tensor_tensor(s0[:, QB + GB1:2 * QB, :],
                                    vfull[:, GB1:QB, 1, :],
                                    tbp[:, GB1:QB, :], op=ALU.mult)
            nc.vector.tensor_reduce(cc[:, q * QB:(q + 1) * QB], s0[:, 0:QB, :],
                                    axis=AX.X, op=ALU.add)
            nc.vector.tensor_reduce(
                cc[:, GRP + q * QB: GRP + (q + 1) * QB],
                s0[:, QB:2 * QB, :], axis=AX.X, op=ALU.add)
            # bf16 copy for stage-3 (gather + u)
            vbf = vbf_pool.tile([128, QB * N], BF16, tag="vbf")
            nc.scalar.copy(vbf[:], venat)
            vbf_g[(g, q)] = vbf

        def emit_stage2(g):
            """softmax + top-16 + bf16 selector build for group g.
            All index-space objects live in n' = j*128 + p order."""
            cc = cc_g[g]
            cmp_ps = ps_tr.tile([128, 256], F32, tag="tr")
            nc.tensor.transpose(cmp_ps[0:GRP, 0:128], cc[:, 0:GRP],
                                ident_f[:])
            nc.tensor.transpose(cmp_ps[0:GRP, 128:256], cc[:, GRP:2 * GRP],
                                ident_f[:])

            dead_i = grp1.tile([GRP, N], I32, tag="deadi")
            nc.sync.dma_start(dead_i[:], dead_d.ap()[g * GRP:(g + 1) * GRP, :])
            dead_f = grp1.tile([GRP, N], F32, tag="deadf")
            nc.vector.tensor_copy(dead_f[:], dead_i[:])
            cm_sb = grp1.tile([GRP, N], F32, tag="cmsb")
            # dead is in raw n order; view it in n' = (j, p) order
            nc.vector.scalar_tensor_tensor(
                cm_sb[:].rearrange("g (j p) -> g j p", j=2),
                dead_f[:].rearrange("g (p j) -> g j p", j=2),
                NEG_MASK,
                cmp_ps[0:GRP, :].rearrange("g (j p) -> g j p", j=2),
                op0=ALU.mult, op1=ALU.add)

            mx_neg = small.tile([GRP, 1], F32, tag="mxneg")
            nc.vector.tensor_reduce(mx_neg[:], cm_sb[:], axis=AX.X, op=ALU.max,
                                    negate=True)
            score_un = grp1.tile([GRP, N], F32, tag="scoreun")
            ssum = small.tile([GRP, 1], F32, tag="ssum")
            nc.scalar.activation(score_un[:], cm_sb[:], ACTF.Exp,
                                 bias=mx_neg[:], scale=1.0, accum_out=ssum[:])
            rs = small.tile([GRP, 1], F32, tag="rs")
            nc.vector.reciprocal(rs[:], ssum[:])
            score_bf = grp1.tile([GRP, N], BF16, tag="scorebf")
            nc.vector.tensor_scalar_mul(score_bf[:], score_un[:], rs[:])

            # top-16 (two rounds of max8 + find_index8), idx in n' space
            mx8a = small.tile([GRP, 8], F32, tag="mx8a")
            nc.vector.max(mx8a[:], cm_sb[:])
            idx16 = small.tile([GRP, K16], U16, tag="idx16")
            nc.vector.max_index(idx16[:, 0:8], mx8a[:], cm_sb[:])
            cm2 = grp1.tile([GRP, N], F32, tag="cm2")
            nc.vector.match_replace(cm2[:], mx8a[:], cm_sb[:], NEG_REPL)
            mx8b = small.tile([GRP, 8], F32, tag="mx8b")
            nc.vector.max(mx8b[:], cm2[:])
            nc.vector.max_index(idx16[:, 8:16], mx8b[:], cm2[:])
            idx_bf = small.tile([GRP, K16], BF16, tag="idxbf")
            nc.vector.tensor_copy(idx_bf[:], idx16[:])
            # flatten idx rows onto one partition via SBUF->SBUF DMA
            idx_pack = tpk_pool.tile([1, GRP * K16], BF16, tag="idxpack")
            nc.sync.dma_start(
                idx_pack[:].rearrange("p (b k) -> p b k", k=K16), idx_bf[:])
            # broadcast indices to all partitions: [128, (ba, j)]
            idx_ps = ps_tbs.tile([128, 1024], F32, tag="tbs")
            nc.tensor.matmul(idx_ps[:, 0:512], ones1_bf[:], idx_pack[:, 0:512],
                             start=True, stop=True)
            nc.tensor.matmul(idx_ps[:, 512:1024], ones1_bf[:],
                             idx_pack[:, 512:1024], start=True, stop=True)
            idx_sb = tbs_pool.tile([128, GRP * K16], BF16, tag="idxsb")
            nc.scalar.copy(idx_sb[:], idx_ps[:])
            # selectors: s[p, ba, j] = (idx[ba, j] == n'(p)) ; col 17 = score
            s_a = selp.tile([128, GRP * (K16 + 1)], BF16, tag="sa")
            s_b = selp.tile([128, GRP * (K16 + 1)], BF16, tag="sb")
            s_a_v = s_a[:].rearrange("p (b j) -> p b j", j=K16 + 1)
            s_b_v = s_b[:].rearrange("p (b j) -> p b j", j=K16 + 1)
            idx_v = idx_sb[:].rearrange("p (b j) -> p b j", j=K16)
            nc.vector.tensor_scalar(s_a_v[:, :, 0:K16], idx_v, iota_p[:], None,
                                    op0=ALU.is_equal)
            nc.vector.tensor_scalar(s_b_v[:, :, 0:K16], idx_v, iota_p2[:], None,
                                    op0=ALU.is_equal)
            # score columns: transpose [ba, n'] -> [n', ba] (bf16)
            st_f = ps_tr.tile([128, 256], F32, tag="tr")
            st_ps = st_f[:].bitcast(BF16)[:, 0:256]
            nc.tensor.transpose(st_ps[0:128, 0:GRP], score_bf[:, 0:128],
                                ident_bf[0:GRP, 0:GRP])
            nc.tensor.transpose(st_ps[0:128, GRP:2 * GRP], score_bf[:, 128:256],
                                ident_bf[0:GRP, 0:GRP])
            nc.scalar.copy(s_a_v[:, :, K16], st_ps[0:128, 0:GRP])
            nc.scalar.copy(s_b_v[:, :, K16], st_ps[0:128, GRP:2 * GRP])
            sel_g[g] = (s_a, s_b)
            xq_g[g] = grp_pool.tile([128, NCH * 128], BF16, tag="xq", name="xq")

        def emit_stage3_chunk(g, c):
            """gather+u for ba in [c*CH, (c+1)*CH): sel-stationary bf16 MMs,
            4 ba packed per psum tile via col tile_position, one PE
            transpose restores [h, (ba-chunk cols)]."""
            s_a, s_b = sel_g[g]
            par = c % 2
            xps = xps_all[:][:, par * 128:(par + 1) * 128]
            for phase in range(2):
                for q4 in range(CH):
                    ba = c * CH + q4
                    vb = vbf_g[(g, ba // QB)]
                    base = (ba % QB) * N
                    lo, hi = ba * 17, (ba + 1) * 17
                    if phase == 0:
                        nc.tensor.matmul(xps[32 * q4:32 * q4 + 17, :],
                                         s_a[:, lo:hi], vb[:, base:base + 128],
                                         start=True, stop=False,
                                         tile_position=(0, 32 * q4))
                    else:
                        nc.tensor.matmul(xps[32 * q4:32 * q4 + 17, :],
                                         s_b[:, lo:hi],
                                         vb[:, base + 128:base + 256],
                                         start=False, stop=True,
                                         tile_position=(0, 32 * q4))
            x_sb = xsb_pool.tile([128, 128], BF16, tag="xsb")
            nc.scalar.copy(x_sb[:], xps)
            xt_ps = xt_all[:][:, par * 128:(par + 1) * 128]
            nc.tensor.transpose(xt_ps, x_sb[:], ident_bf[:])
            nc.vector.tensor_copy(xq_g[g][:, c * 128:(c + 1) * 128], xt_ps)

        def emit_heads(g):
            """vC / vM heads for group g. xq col = c*128 + 32*q4 + j."""
            xq = xq_g[g]
            xq_v = xq[:].rearrange("p (c q w) -> p c q w", q=CH, w=32)
            vc_ps = ps_tr.tile([128, 256], F32, tag="tr")
            nc.tensor.matmul(vc_ps[:, 0:GRP], wf0T[:], vst_f[g][:],
                             start=True, stop=False)
            for j in range(1, K16 + 1):
                nc.tensor.matmul(vc_ps[:, 0:GRP],
                                 wf_bf[:, (j - 1) * H:j * H],
                                 xq_v[:, :, :, j - 1],
                                 start=False, stop=(j == K16))
            vc_sb = grp1.tile([128, GRP], F32, tag="vcsb")
            nc.scalar.activation(vc_sb[:], vc_ps[:, 0:GRP], ACTF.Relu,
                                 bias=bfwd[:], scale=1.0)
            vc_rows = grp1.tile([GRP, H], F32, tag="vcrows")
            pe_transpose(vc_rows[:], vc_sb[:])
            nc.scalar.dma_start(vc_d.ap()[g * GRP:(g + 1) * GRP, :], vc_rows[:])

            vm_ps = ps_tr.tile([128, 256], F32, tag="tr")
            nc.tensor.matmul(vm_ps[:, 0:GRP], wm0T[:], vst_f[g][:],
                             start=True, stop=False)
            nc.tensor.matmul(vm_ps[:, 0:GRP], wmv_bf[:], xq_v[:, :, :, K16],
                             start=False, stop=True)
            vm_sb = grp1.tile([128, GRP], F32, tag="vmsb")
            nc.scalar.activation(vm_sb[:], vm_ps[:, 0:GRP], ACTF.Relu,
                                 bias=bmot[:], scale=1.0)
            vm_rows = grp1.tile([GRP, H], F32, tag="vmrows")
            pe_transpose(vm_rows[:], vm_sb[:])
            nc.scalar.dma_start(vm_d.ap()[g * GRP:(g + 1) * GRP, :], vm_rows[:])

        # ---- software-pipelined emission ----
        emit_tprep(0)
        for q in range(NQ):
            emit_stage1_qb(0, q)
        emit_late_weights()
        for g in range(NG):
            emit_stage2(g)
            if g + 1 < NG:
                emit_tprep(g + 1)
            for q in range(NQ):
                emit_stage3_chunk(g, 2 * q)
                emit_stage3_chunk(g, 2 * q + 1)
                if g + 1 < NG:
                    emit_stage1_qb(g + 1, q)
            emit_heads(g)
            for q in range(NQ):
                del vbf_g[(g, q)]


def _get_compiled(B_pc):
    key = B_pc
    if key not in _CACHE:
        nc = bacc.Bacc("TRN2", target_bir_lowering=False, debug=False,
                       num_devices=N_CORES)
        _build(nc, B_pc)
        nc.compile()
        _CACHE[key] = nc
    return _CACHE[key]


def kernel(vs, ve, ve_dead, Wq, Wk, Wv, W_mot, b_mot, W_fwd, b_fwd,
           trace=False, trace_kwargs=None):
    vs = np.asarray(vs, dtype=np.float32)
    ve = np.asarray(ve, dtype=np.float32)
    ve_dead = np.asarray(ve_dead, dtype=np.int32)
    Bq, Aq = vs.shape[0], vs.shape[1]
    assert (Bq, Aq) == (B, A), (Bq, Aq)
    B_pc = B // N_CORES
    NBA = B_pc * A

    nc = _get_compiled(B_pc)

    shared = {
        "wq": np.ascontiguousarray(Wq, dtype=np.float32),
        "wk": np.ascontiguousarray(Wk, dtype=np.float32),
        "wv": np.ascontiguousarray(Wv, dtype=np.float32),
        "wmot": np.ascontiguousarray(W_mot, dtype=np.float32),
        "bmot": np.ascontiguousarray(b_mot, dtype=np.float32).reshape(H, 1),
        "wfwd": np.ascontiguousarray(W_fwd, dtype=np.float32),
        "bfwd": np.ascontiguousarray(b_fwd, dtype=np.float32).reshape(H, 1),
    }
    in_maps = []
    for c in range(N_CORES):
        sl = slice(c * B_pc, (c + 1) * B_pc)
        in_maps.append({
            "ve": np.ascontiguousarray(ve[sl].reshape(NBA, N, H)),
            "vs": np.ascontiguousarray(vs[sl].reshape(NBA, H)),
            "dead": np.ascontiguousarray(ve_dead[sl].reshape(NBA, N)),
            **shared,
        })

    res = bass_utils.run_bass_kernel_spmd(
        nc, in_maps, core_ids=list(range(N_CORES)),
        trace=trace, **(trace_kwargs or {}))

    vc = np.empty((B, A, H), dtype=np.float32)
    vm = np.empty((B, A, H), dtype=np.float32)
    for c in range(N_CORES):
        sl = slice(c * B_pc, (c + 1) * B_pc)
        vc[sl] = res.results[c]["vc"].reshape(B_pc, A, H)
        vm[sl] = res.results[c]["vm"].reshape(B_pc, A, H)
    kernel.last_results = res
    return (vc, vm)


# revision 24
# speedup vs baseline: 1.0255x; 1.0225x over previous
"""Trainium2 Bass kernel for nn_Concentration_61229053772314.

kernel(**inputs) takes the FULL inputs (B=64), shards the batch dim across
8 NeuronCores (pure data parallel, weights replicated), runs a Bass/Tile
kernel via run_bass_kernel_spmd, and reassembles the full outputs.

v5 architecture (per core: NBA=256 (b,a) rows, 4 groups of GRP=64):
 - ve streamed once as f32 in parity layout [p, (b, j, h)] with n = 2p+j:
   1KB-contiguous DMA descriptors (two n-rows per partition).  All
   index-space objects (compat cols, topk idx, selectors, score) live in
   the permuted space n' = j*128 + p; only the dead-mask load needs a
   strided view.  venat f32 is transient: consumed by stage-1 + cast to
   a bf16 copy for stage-3, then freed.
 - Queue split: the sync ring carries ONLY the 1MB venat streams (no
   head-of-line blocking); every small DMA goes on the scalar ring.
 - compat = sum_h ve*t: t broadcast via one K=2 PE matmul per 512 cols
   (hi/lo f32r planes stacked on adjacent partitions, exact f32 sum);
   multiply split gpsimd/DVE; DVE scratch+tbs live in PSUM to relieve
   the shared gpsimd/DVE SBUF port; gpsimd reduces its own j1 share.
 - t packs batched: one [16, 1024] tile per group holds all 8 steps'
   hi/lo planes on partition pairs (1 DMA per group).
 - stage-3 gather: selector is the 17-col STATIONARY, ve_bf16 moving
   (1 cyc/row); 4 ba per PSUM tile via col tile_position; one PE
   transpose per chunk restores [h, (ba,j)].  Gathered values / u are
   bf16 (tol 2e-2); compat/top-k stay f32.
 - heads: W_fwd blocks 1..16 and W_mot@WvT in bf16; vs-terms exact f32.
"""
import math
import os
import sys

for _p in ("/opt/trn_rl_repo", "/root/.axon_site/_ro/trn_rl_repo"):
    if os.path.isdir(_p) and _p not in sys.path:
        sys.path.insert(0, _p)

import numpy as np
import concourse.tile as tile
from concourse import bacc, bass_utils, mybir

F32 = mybir.dt.float32
F32R = mybir.dt.float32r
BF16 = mybir.dt.bfloat16
I32 = mybir.dt.int32
U16 = mybir.dt.uint16
AX = mybir.AxisListType
ALU = mybir.AluOpType
ACTF = mybir.ActivationFunctionType

N_CORES = 8
B, A = 64, 32
N = 256    # entries per (b,a)
H = 128    # head dim
K16 = 16   # top-k
GRP = 64   # (b,a) pairs per processing group
QB = 8     # ba per DMA / pipeline step
NQ = GRP // QB  # steps per group (8)
CH = 4     # ba per stage-3 psum chunk
NCH = GRP // CH  # chunks per group (16)
GB1 = 3    # j1 b-columns multiplied on gpsimd

NEG_MASK = -1.0e30   # added to masked entries
NEG_REPL = -3.0e38   # match_replace fill (below any real/masked value)

_CACHE = {}


def _build(nc, B_pc):
    NBA = 32 * B_pc
    assert NBA % GRP == 0
    NG = NBA // GRP

    ve_d = nc.dram_tensor("ve", [NBA, N, H], F32, kind="ExternalInput")
    vs_d = nc.dram_tensor("vs", [NBA, H], F32, kind="ExternalInput")
    dead_d = nc.dram_tensor("dead", [NBA, N], I32, kind="ExternalInput")
    wq_d = nc.dram_tensor("wq", [H, H], F32, kind="ExternalInput")
    wk_d = nc.dram_tensor("wk", [H, H], F32, kind="ExternalInput")
    wv_d = nc.dram_tensor("wv", [H, H], F32, kind="ExternalInput")
    wmot_d = nc.dram_tensor("wmot", [H, 2 * H], F32, kind="ExternalInput")
    bmot_d = nc.dram_tensor("bmot", [H, 1], F32, kind="ExternalInput")
    wfwd_d = nc.dram_tensor("wfwd", [H, (K16 + 1) * H], F32, kind="ExternalInput")
    bfwd_d = nc.dram_tensor("bfwd", [H, 1], F32, kind="ExternalInput")
    vc_d = nc.dram_tensor("vc", [NBA, H], F32, kind="ExternalOutput")
    vm_d = nc.dram_tensor("vm", [NBA, H], F32, kind="ExternalOutput")

    with tile.TileContext(nc) as tc:
        _body(nc, tc, NBA, NG, ve_d, vs_d, dead_d, wq_d, wk_d, wv_d,
              wmot_d, bmot_d, wfwd_d, bfwd_d, vc_d, vm_d)


def _body(nc, tc, NBA, NG, ve_d, vs_d, dead_d, wq_d, wk_d, wv_d,
          wmot_d, bmot_d, wfwd_d, bfwd_d, vc_d, vm_d):
    from contextlib import ExitStack
    with ExitStack() as ctx:
        consts = ctx.enter_context(tc.tile_pool(name="consts", bufs=1))
        wres = ctx.enter_context(tc.tile_pool(name="wres", bufs=1))
        venat_pool = ctx.enter_context(tc.tile_pool(name="venat", bufs=2))
        vbf_pool = ctx.enter_context(tc.tile_pool(name="vbf", bufs=10))
        scr_pool = ctx.enter_context(tc.tile_pool(name="scr", bufs=2))
        tbs_pool = ctx.enter_context(tc.tile_pool(name="tbs", bufs=2))
        tpk_pool = ctx.enter_context(tc.tile_pool(name="tpk", bufs=2))
        tpre = ctx.enter_context(tc.tile_pool(name="tpre", bufs=2))
        vst_pool = ctx.enter_context(tc.tile_pool(name="vst", bufs=2))
        grp_pool = ctx.enter_context(tc.tile_pool(name="grp", bufs=2))
        grp1 = ctx.enter_context(tc.tile_pool(name="grp1", bufs=2))
        selp = ctx.enter_context(tc.tile_pool(name="selp", bufs=1))
        xsb_pool = ctx.enter_context(tc.tile_pool(name="xsb", bufs=3))
        small = ctx.enter_context(tc.tile_pool(name="small", bufs=3))
        dram_pool = ctx.enter_context(tc.tile_pool(name="dram", bufs=4, space="DRAM"))
        # PSUM budget, bank-granular (8 banks of 2KB/partition):
        #   ps_tbs [128,1024]f32 x2            = 4 banks
        #   ps_tr  [128,256]f32 x1             = 1 bank

        #   ps_x   [128,256]f32 persistent     = 1 bank
        #   ps_xt  [128,256]bf16 persistent    = 1 bank
        ps_tbs = ctx.enter_context(tc.tile_pool(name="ps_tbs", bufs=2, space="PSUM"))
        ps_tr = ctx.enter_context(tc.tile_pool(name="ps_tr", bufs=1, space="PSUM"))
        ps_x = ctx.enter_context(tc.tile_pool(name="ps_x", bufs=1, space="PSUM"))
        ps_xt = ctx.enter_context(tc.tile_pool(name="ps_xt", bufs=1, space="PSUM"))

        # ---- constants ----
        iota_n = consts.tile([128, 128], I32)
        nc.gpsimd.iota(iota_n[:], pattern=[[1, 128]], base=0, channel_multiplier=0)
        iota_p = consts.tile([128, 1], F32)
        nc.gpsimd.iota(iota_p[:], pattern=[[0, 1]], base=0, channel_multiplier=1,
                       allow_small_or_imprecise_dtypes=True)
        iota_p2 = consts.tile([128, 1], F32)  # p + 128
        nc.gpsimd.iota(iota_p2[:], pattern=[[0, 1]], base=128, channel_multiplier=1,
                       allow_small_or_imprecise_dtypes=True)
        ident_f = consts.tile([128, 128], F32)
        nc.vector.tensor_scalar(ident_f[:], iota_n[:], iota_p[:], None,
                                op0=ALU.is_equal)
        ident_bf = consts.tile([128, 128], BF16)
        nc.vector.tensor_copy(ident_bf[:], ident_f[:])
        ident_r = consts.tile([128, 128], F32R)
        nc.scalar.copy(ident_r[:], ident_f[:])
        ones2_f = consts.tile([2, 128], F32)
        nc.gpsimd.memset(ones2_f[:], 1.0)
        ones2_r = consts.tile([2, 128], F32R)
        nc.scalar.copy(ones2_r[:], ones2_f[:])
        ones1_bf = consts.tile([1, 128], BF16)
        nc.gpsimd.memset(ones1_bf[:], 1.0)

        def pe_transpose(dst_sb, src_sb, eng=nc.scalar):
            """dst[f, p] = src[p, f] via PE; dst in SBUF (f32 path)."""
            p_in, f_in = src_sb.shape[0], src_sb.shape[1]
            ps = ps_tr.tile([128, 256], F32, tag="tr")
            out = ps[0:f_in, 0:p_in]
            nc.tensor.transpose(out, src_sb, ident_f[0:p_in, 0:p_in])
            eng.copy(dst_sb, out)

        # ---- early weights: only what tprep needs (wq, wkT) ----
        with tc.tile_pool(name="wtmp0", bufs=1) as wtmp0:
            wq = wres.tile([H, H], F32)
            nc.scalar.dma_start(wq[:], wq_d.ap())
            wk = wtmp0.tile([H, H], F32)
            nc.scalar.dma_start(wk[:], wk_d.ap())
            wkT = wres.tile([H, H], F32)
            pe_transpose(wkT[:], wk[:])
        bmot = wres.tile([H, 1], F32)
        nc.scalar.dma_start(bmot[:], bmot_d.ap())
        bfwd = wres.tile([H, 1], F32)
        nc.scalar.dma_start(bfwd[:], bfwd_d.ap())
        wm0T = wres.tile([H, H], F32)
        wmv_bf = wres.tile([H, H], BF16)
        wf0T = wres.tile([H, H], F32)
        wf_bf = wres.tile([H, K16 * H], BF16)

        def emit_late_weights():
            """head weights: emitted after stage1(0) so the transposes
            overlap the streaming pipeline instead of delaying it."""
            with tc.tile_pool(name="wtmp", bufs=1) as wtmp:
                wv = wtmp.tile([H, H], F32)
                nc.scalar.dma_start(wv[:], wv_d.ap())
                wmot = wtmp.tile([H, 2 * H], F32)
                nc.scalar.dma_start(wmot[:], wmot_d.ap())
                wfwd = wtmp.tile([H, (K16 + 1) * H], F32)
                nc.scalar.dma_start(wfwd[:], wfwd_d.ap())
                wvT = wtmp.tile([H, H], F32)
                pe_transpose(wvT[:], wv[:])
                pe_transpose(wm0T[:], wmot[:, 0:H])
                wm1T = wtmp.tile([H, H], F32)
                pe_transpose(wm1T[:], wmot[:, H:2 * H])
                wmvT_f = wtmp.tile([H, H], F32)
                ps = ps_tr.tile([128, 256], F32, tag="tr")
                nc.tensor.matmul(ps[:, 0:128], wvT[:], wm1T[:])
                nc.scalar.copy(wmvT_f[:], ps[:, 0:128])
                nc.scalar.copy(wmv_bf[:], wmvT_f[:])
                pe_transpose(wf0T[:], wfwd[:, 0:H])
                for j in range(1, K16 + 1):
                    pe_transpose(wf_bf[:, (j - 1) * H:j * H],
                                 wfwd[:, j * H:(j + 1) * H])

        # ---- per-group state ----
        xps_all = ps_x.tile([128, 256], F32, tag="x")
        vpair_box = [None]
        xt_all = ps_xt.tile([128, 256], BF16, tag="xt")
        vst_f = {}       # g -> vs^T tile [H, GRP] f32
        tpk16_g = {}     # g -> [16, QB*H] f32r (hi/lo planes per step)
        vbf_g = {}       # (g, t8) -> bf16 venat tile [128, QB*N]
        cc_g = {}        # g -> cc tile [128, 2*GRP] ([p, (j, ba)])
        sel_g = {}       # g -> (s_a, s_b) bf16 [128, GRP*17]
        xq_g = {}        # g -> gathered tile [128, NCH*128] bf16

        def emit_tprep(g):
            """t = (Wk^T Wq^T vs)/sqrt(H) rows, split hi/lo f32r, batched
            into one [16, QB*H] pack per group via a DRAM bounce."""
            vs_rows = tpre.tile([GRP, H], F32, tag="vsrows")
            nc.scalar.dma_start(vs_rows[:], vs_d.ap()[g * GRP:(g + 1) * GRP, :])
            vstf = vst_pool.tile([H, GRP], F32, tag="vstf")
            pe_transpose(vstf[:], vs_rows[:])
            qt = tpre.tile([H, GRP], F32, tag="qt")
            ps = ps_tr.tile([128, 256], F32, tag="tr")
            nc.tensor.matmul(ps[:, 0:GRP], wq[:], vstf[:])
            nc.scalar.copy(qt[:], ps[:, 0:GRP])
            tsb = tpre.tile([H, GRP], F32, tag="tsb")
            ps = ps_tr.tile([128, 256], F32, tag="tr")
            nc.tensor.matmul(ps[:, 0:GRP], wkT[:], qt[:])
            nc.scalar.mul(tsb[:], ps[:, 0:GRP], 1.0 / math.sqrt(H))
            trows_f = tpre.tile([GRP, H], F32, tag="trowsf")
            pe_transpose(trows_f[:], tsb[:])
            trows_r = tpre.tile([GRP, H], F32R, tag="trowsr")
            nc.scalar.copy(trows_r[:], trows_f[:])
            tlo_r = tpre.tile([GRP, H], F32R, tag="tlor")
            nc.vector.tensor_tensor(tlo_r[:], trows_f[:], trows_r[:].bitcast(F32),
                                    op=ALU.subtract)
            t_dram = dram_pool.tile([2, GRP, H], F32R, tag="tdram")
            nc.scalar.dma_start(t_dram[:][0], trows_r[:])
            nc.scalar.dma_start(t_dram[:][1], tlo_r[:])
            # packs of 2 steps: [2 planes, (q2, b, h)]
            packs = []
            for half in range(NQ // 2):
                tpk4 = tpk_pool.tile([2, 2 * QB * H], F32R, tag="tpk4")
                nc.scalar.dma_start(
                    tpk4[:].rearrange("pl (q b h) -> pl q b h", q=2, b=QB),
                    t_dram[:].rearrange("pl (q b) h -> pl q b h", b=QB)
                    [:, half * 2:(half + 1) * 2])
                packs.append(tpk4)
            vst_f[g] = vstf
            tpk16_g[g] = packs

        def emit_stage1_qb(g, q):
            """load QB ba's of ve (parity layout), broadcast t,
            multiply+reduce, cast bf16."""
            ib = g * GRP + q * QB
            if q == 0:
                cc_g[g] = grp_pool.tile([128, 2 * GRP], F32, tag="cc", name="cc")
            cc = cc_g[g]
            if q % 4 == 0:
                vpair = venat_pool.tile([128, 4 * QB * N], F32, tag="venat")
                # four 1MB slice loads: consumers unblock per-slice while
                # the ring streams at its (size-independent) rate
                for k in range(4):
                    srck = ve_d.ap()[ib + k * QB:ib + (k + 1) * QB].rearrange(
                        "b (p j) h -> p b j h", j=2)
                    dstk = (vpair[:][:, k * QB * N:(k + 1) * QB * N]
                            .rearrange("p (b j h) -> p b j h", b=QB, j=2))
                    nc.sync.dma_start(dstk, srck)
                vpair_box[0] = vpair
            venat = vpair_box[0][:][:, (q % 4) * QB * N:(q % 4 + 1) * QB * N]
            # broadcast t across partitions: K=2 matmul sums hi+lo exactly
            tpk4 = tpk16_g[g][q // 2]
            qo = (q % 2) * QB * H
            tbs_ps = ps_tbs.tile([128, QB * H], F32, tag="tbs")
            nc.tensor.matmul(tbs_ps[:, 0:512],
                             ones2_r[:], tpk4[:, qo:qo + 512],
                             start=True, stop=True)
            nc.tensor.matmul(tbs_ps[:, 512:1024],
                             ones2_r[:], tpk4[:, qo + 512:qo + 1024],
                             start=True, stop=True)
            tbs = tbs_pool.tile([128, QB * H], F32, tag="tbs")
            nc.scalar.copy(tbs[:], tbs_ps[:])
            vfull = venat.rearrange("p (b j h) -> p b j h", b=QB, j=2)
            tb = tbs[:].rearrange("p (b h) -> p b h", b=QB)
            tbp = tbs_ps[:].rearrange("p (b h) -> p b h", b=QB)
            # gpsimd: j0 all b + j1 b[0:GB1] (mult+reduce); DVE: j1 b[GB1:]
            # with PSUM tbs + PSUM scratch (keeps the shared SBUF port free)
            scr = scr_pool.tile([128, 2 * QB * H], F32, tag="scr")
            s0 = scr[:].rearrange("p (b h) -> p b h", b=2 * QB)
            nc.gpsimd.tensor_tensor(s0[:, 0:QB, :], vfull[:, :, 0, :], tb,
                                    op=ALU.mult)
            nc.gpsimd.tensor_tensor(s0[:, QB:QB + GB1, :], vfull[:, 0:GB1, 1, :],
                                    tb[:, 0:GB1, :], op=ALU.mult)
            nc.vector.tensor_tensor(s0[:, QB + GB1:2 * QB, :],
                                    vfull[:, GB1:QB, 1, :],
                                    tbp[:, GB1:QB, :], op=ALU.mult)
            nc.vector.tensor_reduce(cc[:, q * QB:(q + 1) * QB], s0[:, 0:QB, :],
                                    axis=AX.X, op=ALU.add)
            nc.vector.tensor_reduce(
                cc[:, GRP + q * QB: GRP + (q + 1) * QB],
                s0[:, QB:2 * QB, :], axis=AX.X, op=ALU.add)
            # bf16 copy for stage-3 (gather + u)
            vbf = vbf_pool.tile([128, QB * N], BF16, tag="vbf")
            nc.scalar.copy(vbf[:], venat)
            vbf_g[(g, q)] = vbf

        def emit_stage2(g):
            """softmax + top-16 + bf16 selector build for group g.
            All index-space objects live in n' = j*128 + p order."""
            cc = cc_g[g]
            cmp_ps = ps_tr.tile([128, 256], F32, tag="tr")
            nc.tensor.transpose(cmp_ps[0:GRP, 0:128], cc[:, 0:GRP],
                                ident_f[:])
            nc.tensor.transpose(cmp_ps[0:GRP, 128:256], cc[:, GRP:2 * GRP],
                                ident_f[:])

            dead_i = grp1.tile([GRP, N], I32, tag="deadi")
            nc.sync.dma_start(dead_i[:], dead_d.ap()[g * GRP:(g + 1) * GRP, :])
            dead_f = grp1.tile([GRP, N], F32, tag="deadf")
            nc.vector.tensor_copy(dead_f[:], dead_i[:])
            cm_sb = grp1.tile([GRP, N], F32, tag="cmsb")
            # dead is in raw n order; view it in n' = (j, p) order
            nc.vector.scalar_tensor_tensor(
                cm_sb[:].rearrange("g (j p) -> g j p", j=2),
                dead_f[:].rearrange("g (p j) -> g j p", j=2),
                NEG_MASK,
                cmp_ps[0:GRP, :].rearrange("g (j p) -> g j p", j=2),
                op0=ALU.mult, op1=ALU.add)

            mx_neg = small.tile([GRP, 1], F32, tag="mxneg")
            nc.vector.tensor_reduce(mx_neg[:], cm_sb[:], axis=AX.X, op=ALU.max,
                                    negate=True)
            score_un = grp1.tile([GRP, N], F32, tag="scoreun")
            ssum = small.tile([GRP, 1], F32, tag="ssum")
            nc.scalar.activation(score_un[:], cm_sb[:], ACTF.Exp,
                                 bias=mx_neg[:], scale=1.0, accum_out=ssum[:])
            rs = small.tile([GRP, 1], F32, tag="rs")
            nc.vector.reciprocal(rs[:], ssum[:])
            score_bf = grp1.tile([GRP, N], BF16, tag="scorebf")
            nc.vector.tensor_scalar_mul(score_bf[:], score_un[:], rs[:])

            # top-16 (two rounds of max8 + find_index8), idx in n' space
            mx8a = small.tile([GRP, 8], F32, tag="mx8a")
            nc.vector.max(mx8a[:], cm_sb[:])
            idx16 = small.tile([GRP, K16], U16, tag="idx16")
            nc.vector.max_index(idx16[:, 0:8], mx8a[:], cm_sb[:])
            cm2 = grp1.tile([GRP, N], F32, tag="cm2")
            nc.vector.match_replace(cm2[:], mx8a[:], cm_sb[:], NEG_REPL)
            mx8b = small.tile([GRP, 8], F32, tag="mx8b")
            nc.vector.max(mx8b[:], cm2[:])
            nc.vector.max_index(idx16[:, 8:16], mx8b[:], cm2[:])
            idx_bf = small.tile([GRP, K16], BF16, tag="idxbf")
            nc.vector.tensor_copy(idx_bf[:], idx16[:])
            # flatten idx rows onto one partition via SBUF->SBUF DMA
            idx_pack = tpk_pool.tile([1, GRP * K16], BF16, tag="idxpack")
            nc.sync.dma_start(
                idx_pack[:].rearrange("p (b k) -> p b k", k=K16), idx_bf[:])
            # broadcast indices to all partitions: [128, (ba, j)]
            idx_ps = ps_tbs.tile([128, 1024], F32, tag="tbs")
            nc.tensor.matmul(idx_ps[:, 0:512], ones1_bf[:], idx_pack[:, 0:512],
                             start=True, stop=True)
            nc.tensor.matmul(idx_ps[:, 512:1024], ones1_bf[:],
                             idx_pack[:, 512:1024], start=True, stop=True)
            idx_sb = tbs_pool.tile([128, GRP * K16], BF16, tag="idxsb")
            nc.scalar.copy(idx_sb[:], idx_ps[:])
            # selectors: s[p, ba, j] = (idx[ba, j] == n'(p)) ; col 17 = score
            s_a = selp.tile([128, GRP * (K16 + 1)], BF16, tag="sa")
            s_b = selp.tile([128, GRP * (K16 + 1)], BF16, tag="sb")
            s_a_v = s_a[:].rearrange("p (b j) -> p b j", j=K16 + 1)
            s_b_v = s_b[:].rearrange("p (b j) -> p b j", j=K16 + 1)
            idx_v = idx_sb[:].rearrange("p (b j) -> p b j", j=K16)
            nc.vector.tensor_scalar(s_a_v[:, :, 0:K16], idx_v, iota_p[:], None,
                                    op0=ALU.is_equal)
            nc.vector.tensor_scalar(s_b_v[:, :, 0:K16], idx_v, iota_p2[:], None,
                                    op0=ALU.is_equal)
            # score columns: transpose [ba, n'] -> [n', ba] (bf16)
            st_f = ps_tr.tile([128, 256], F32, tag="tr")
            st_ps = st_f[:].bitcast(BF16)[:, 0:256]
            nc.tensor.transpose(st_ps[0:128, 0:GRP], score_bf[:, 0:128],
                                ident_bf[0:GRP, 0:GRP])
            nc.tensor.transpose(st_ps[0:128, GRP:2 * GRP], score_bf[:, 128:256],
                                ident_bf[0:GRP, 0:GRP])
            nc.scalar.copy(s_a_v[:, :, K16], st_ps[0:128, 0:GRP])
            nc.scalar.copy(s_b_v[:, :, K16], st_ps[0:128, GRP:2 * GRP])
            sel_g[g] = (s_a, s_b)
            xq_g[g] = grp_pool.tile([128, NCH * 128], BF16, tag="xq", name="xq")

        def emit_stage3_chunk(g, c):
            """gather+u for ba in [c*CH, (c+1)*CH): sel-stationary bf16 MMs,
            4 ba packed per psum tile via col tile_position, one PE
            transpose restores [h, (ba-chunk cols)]."""
            s_a, s_b = sel_g[g]
            par = c % 2
            xps = xps_all[:][:, par * 128:(par + 1) * 128]
            for phase in range(2):
                for q4 in range(CH):
                    ba = c * CH + q4
                    vb = vbf_g[(g, ba // QB)]
                    base = (ba % QB) * N
                    lo, hi = ba * 17, (ba + 1) * 17
                    if phase == 0:
                        nc.tensor.matmul(xps[32 * q4:32 * q4 + 17, :],
                                         s_a[:, lo:hi], vb[:, base:base + 128],
                                         start=True, stop=False,
                                         tile_position=(0, 32 * q4))
                    else:
                        nc.tensor.matmul(xps[32 * q4:32 * q4 + 17, :],
                                         s_b[:, lo:hi],
                                         vb[:, base + 128:base + 256],
                                         start=False, stop=True,
                                         tile_position=(0, 32 * q4))
            x_sb = xsb_pool.tile([128, 128], BF16, tag="xsb")
            nc.scalar.copy(x_sb[:], xps)
            xt_ps = xt_all[:][:, par * 128:(par + 1) * 128]
            nc.tensor.transpose(xt_ps, x_sb[:], ident_bf[:])
            nc.vector.tensor_copy(xq_g[g][:, c * 128:(c + 1) * 128], xt_ps)

        def emit_heads(g):
            """vC / vM heads for group g. xq col = c*128 + 32*q4 + j."""
            xq = xq_g[g]
            xq_v = xq[:].rearrange("p (c q w) -> p c q w", q=CH, w=32)
            vc_ps = ps_tr.tile([128, 256], F32, tag="tr")
            nc.tensor.matmul(vc_ps[:, 0:GRP], wf0T[:], vst_f[g][:],
                             start=True, stop=False)
            for j in range(1, K16 + 1):
                nc.tensor.matmul(vc_ps[:, 0:GRP],
                                 wf_bf[:, (j - 1) * H:j * H],
                                 xq_v[:, :, :, j - 1],
                                 start=False, stop=(j == K16))
            vc_sb = grp1.tile([128, GRP], F32, tag="vcsb")
            nc.scalar.activation(vc_sb[:], vc_ps[:, 0:GRP], ACTF.Relu,
                                 bias=bfwd[:], scale=1.0)
            vc_rows = grp1.tile([GRP, H], F32, tag="vcrows")
            pe_transpose(vc_rows[:], vc_sb[:])
            nc.scalar.dma_start(vc_d.ap()[g * GRP:(g + 1) * GRP, :], vc_rows[:])

            vm_ps = ps_tr.tile([128, 256], F32, tag="tr")
            nc.tensor.matmul(vm_ps[:, 0:GRP], wm0T[:], vst_f[g][:],
                             start=True, stop=False)
            nc.tensor.matmul(vm_ps[:, 0:GRP], wmv_bf[:], xq_v[:, :, :, K16],
                             start=False, stop=True)
            vm_sb = grp1.tile([128, GRP], F32, tag="vmsb")
            nc.scalar.activation(vm_sb[:], vm_ps[:, 0:GRP], ACTF.Relu,
                                 bias=bmot[:], scale=1.0)
            vm_rows = grp1.tile([GRP, H], F32, tag="vmrows")
            pe_transpose(vm_rows[:], vm_sb[:])
            nc.scalar.dma_start(vm_d.ap()[g * GRP:(g + 1) * GRP, :], vm_rows[:])

        # ---- software-pipelined emission ----
        emit_tprep(0)
        for q in range(NQ):
            emit_stage1_qb(0, q)
        emit_late_weights()
        for g in range(NG):
            emit_stage2(g)
            if g + 1 < NG:
                emit_tprep(g + 1)
            for q in range(NQ):
                emit_stage3_chunk(g, 2 * q)
                emit_stage3_chunk(g, 2 * q + 1)
                if g + 1 < NG:
                    emit_stage1_qb(g + 1, q)
            emit_heads(g)
            for q in range(NQ):
                del vbf_g[(g, q)]


def _get_compiled(B_pc):
    key = B_pc
    if key not in _CACHE:
        nc = bacc.Bacc("TRN2", target_bir_lowering=False, debug=False,
                       num_devices=N_CORES)
        _build(nc, B_pc)
        nc.compile()
        _CACHE[key] = nc
    return _CACHE[key]


def kernel(vs, ve, ve_dead, Wq, Wk, Wv, W_mot, b_mot, W_fwd, b_fwd,
           trace=False, trace_kwargs=None):
    vs = np.asarray(vs, dtype=np.float32)
    ve = np.asarray(ve, dtype=np.float32)
    ve_dead = np.asarray(ve_dead, dtype=np.int32)
    Bq, Aq = vs.shape[0], vs.shape[1]
    assert (Bq, Aq) == (B, A), (Bq, Aq)
    B_pc = B // N_CORES
    NBA = B_pc * A

    nc = _get_compiled(B_pc)

    shared = {
        "wq": np.ascontiguousarray(Wq, dtype=np.float32),
        "wk": np.ascontiguousarray(Wk, dtype=np.float32),
        "wv": np.ascontiguousarray(Wv, dtype=np.float32),
        "wmot": np.ascontiguousarray(W_mot, dtype=np.float32),
        "bmot": np.ascontiguousarray(b_mot, dtype=np.float32).reshape(H, 1),
        "wfwd": np.ascontiguousarray(W_fwd, dtype=np.float32),
        "bfwd": np.ascontiguousarray(b_fwd, dtype=np.float32).reshape(H, 1),
    }
    in_maps = []
    for c in range(N_CORES):
        sl = slice(c * B_pc, (c + 1) * B_pc)
        in_maps.append({
            "ve": np.ascontiguousarray(ve[sl].reshape(NBA, N, H)),
            "vs": np.ascontiguousarray(vs[sl].reshape(NBA, H)),
            "dead": np.ascontiguousarray(ve_dead[sl].reshape(NBA, N)),
            **shared,
        })

    res = bass_utils.run_bass_kernel_spmd(
        nc, in_maps, core_ids=list(range(N_CORES)),
        trace=trace, **(trace_kwargs or {}))

    vc = np.empty((B, A, H), dtype=np.float32)
    vm = np.empty((B, A, H), dtype=np.float32)
    for c in range(N_CORES):
        sl = slice(c * B_pc, (c + 1) * B_pc)
        vc[sl] = res.results[c]["vc"].reshape(B_pc, A, H)
        vm[sl] = res.results[c]["vm"].reshape(B_pc, A, H)
    kernel.last_results = res
    return (vc, vm)
